# revision 6
# baseline (speedup 1.0000x reference)
"""
Trainium2 Bass kernel for nn_DKNN (differentiable kNN via NeuralSort + PL sampling).

Math (per (sample p, query m) pair, n=1024 neighbors, K=16, tau=1):
    scores[m,n] = -||q_m - nb_n||^2 ; softmax over n is invariant to the
    per-query ||q_m||^2 term, so we use  t = 2 q.nb - ||nb||^2  instead.
    s = t + gumbel                      # [n]
    B_i = sum_j |s_i - s_j|             # O(n^2) -- the hot loop
    l[r,i] = scaling_r * s_i - B_i      # scaling_r = n+1-2(r+1), r=0..15
    out[i] = sum_r softmax_i(l[r,:])    # [n]

Sharding: 64 independent (p,m) pairs -> 8 pairs per NeuronCore.

Engine mapping per core (v2):
    GPS   : partition_broadcast of each pair's s row -> SBUF [128,1024]
    ACT   : fused |bcast - s_i| + accumulate, segmented FD=512 for fp32
            accuracy (B partial sums must not run 1024-long sequentially);
            exp with bias=-rowmax and accumulate -> Z
    DVE   : other strips: tensor_scalar subtract (2x) + segmented abs-reduce
            [128,4,256]; rowmax; reciprocal; P = E * (1/Z); combines
    PE    : scores matmul, s transposes, logits outer-product matmuls
            (s-part early / B-part late), final K-row-sum matmul
"""

import os
import sys

import numpy as np

sys.path.insert(0, "/opt/trn_rl_repo")


def _install_ntff_hook_shim():
    """The agent image's `antenv` lacks `axon_hooks`; provide it so
    run_bass_kernel_spmd(trace=True) can capture NTFF profiles via the
    boot module's ctypes hook."""
    import types

    if "antenv.axon_hooks" in sys.modules:
        return
    mod = types.ModuleType("antenv.axon_hooks")
    state = {"hook": None}
    mod.set_axon_ntff_profile_hook = lambda h: state.__setitem__("hook", h)
    mod.get_axon_ntff_profile_hook = lambda: state["hook"]
    sys.modules["antenv.axon_hooks"] = mod
    try:
        from trn_agent_boot.trn_boot import _ntff_profile_via_ctypes

        mod.set_axon_ntff_profile_hook(
            _ntff_profile_via_ctypes("/opt/axon/libaxon_pjrt.so")
        )
    except Exception:
        pass


_install_ntff_hook_shim()

import concourse.bass as bass
import concourse.mybir as mybir
import concourse.tile as tile
from concourse import bacc
from concourse.bass_utils import run_bass_kernel_spmd

F32 = mybir.dt.float32
AF = mybir.ActivationFunctionType
ALU = mybir.AluOpType
AX = mybir.AxisListType

N = 1024          # neighbors
D = 128           # feature dim
M = 32            # queries
S = 2             # PL samples
K = 16            # top-k
NCORES = 8
PAIRS = 8         # (p, m) pairs per core
NCHUNK = 8        # i-chunks of 128 per pair
HALF = 512        # matmul N <= 512 (one PSUM bank)

# strips (i-chunks) 0..ACT_SPLIT-1 on ScalarE, the rest on VectorE
ACT_SPLIT = int(os.environ.get("DK_ACT_SPLIT", "4"))


def build_nc():
    nc = bacc.Bacc("TRN2", target_bir_lowering=False, debug=False)

    with tile.TileContext(nc) as tc:
        with tc.tile_pool(name="dram", bufs=1, space="DRAM") as dram:
            d_nbT = dram.tile([D, N], F32, kind="ExternalInput", name="nbT", uniquify=False)
            d_qT2 = dram.tile([D, PAIRS], F32, kind="ExternalInput", name="qT2", uniquify=False)
            d_negnb2 = dram.tile([1, N], F32, kind="ExternalInput", name="negnb2", uniquify=False)
            d_gum8 = dram.tile([PAIRS, N], F32, kind="ExternalInput", name="gum8", uniquify=False)
            d_ident = dram.tile([D, D], F32, kind="ExternalInput", name="ident", uniquify=False)
            d_ones8 = dram.tile([1, PAIRS], F32, kind="ExternalInput", name="ones8", uniquify=False)
            d_lhs_s = dram.tile([PAIRS, D], F32, kind="ExternalInput", name="lhs_s", uniquify=False)
            d_lhs_b = dram.tile([PAIRS, D], F32, kind="ExternalInput", name="lhs_b", uniquify=False)
            d_ones8t = dram.tile([D, PAIRS], F32, kind="ExternalOutput" if False else "ExternalInput", name="ones8t", uniquify=False)
            d_out = dram.tile([PAIRS, N], F32, kind="ExternalOutput", name="out", uniquify=False)

            with tc.tile_pool(name="consts", bufs=1) as consts:
                nbT = consts.tile([D, N], F32)
                qT2 = consts.tile([D, PAIRS], F32)
                negnb2 = consts.tile([1, N], F32)
                gum8 = consts.tile([PAIRS, N], F32)
                ident = consts.tile([D, D], F32)
                ones8 = consts.tile([1, PAIRS], F32)
                lhs_s = consts.tile([PAIRS, D], F32)
                lhs_b = consts.tile([PAIRS, D], F32)
                ones8t = consts.tile([D, PAIRS], F32)
                for sb, dr in [
                    (nbT, d_nbT), (qT2, d_qT2), (negnb2, d_negnb2), (gum8, d_gum8),
                    (ident, d_ident), (ones8, d_ones8), (lhs_s, d_lhs_s),
                    (lhs_b, d_lhs_b), (ones8t, d_ones8t),
                ]:
                    nc.sync.dma_start(out=sb[:], in_=dr[:])

                with tc.tile_pool(name="work", bufs=1) as work:
                    s_rows = work.tile([PAIRS, N], F32)
                    nst = work.tile([D, PAIRS * NCHUNK], F32)   # col 8c+pr = -s_pr[128c+p]
                    ptile = work.tile([D, PAIRS * 32], F32)     # col pr*32+c*4+g = partial sums
                    b_col = work.tile([D, PAIRS * NCHUNK], F32)  # col 8pr+c = B_pr[128c+p]
                    b_rows = work.tile([PAIRS, N], F32)
                    bt_sb = work.tile([PAIRS * NCHUNK, D], F32)
                    e_sb = work.tile([D, N], F32)
                    p_sb = work.tile([D, N], F32)
                    negmax = work.tile([D, 1], F32)
                    zden = work.tile([D, 1], F32)
                    invz = work.tile([D, 1], F32)
                    out_sb = work.tile([PAIRS, N], F32)
                    srow = [work.tile([1, N], F32, name=f"srow{i}") for i in range(PAIRS)]

                    nc.gpsimd.memset(ptile[:], 0.0)

                    # ---- s = (2 q.nb - nb2) + gumbel -----------------------------
                    with tc.tile_pool(name="psum_s", bufs=1, space="PSUM") as pp_s:
                        scores8 = pp_s.tile([PAIRS, N], F32)
                        for h in range(2):
                            hs = slice(h * HALF, (h + 1) * HALF)
                            nc.tensor.matmul(scores8[:, hs], qT2[:], nbT[:, hs],
                                             start=True, stop=False)
                            nc.tensor.matmul(scores8[:, hs], ones8[:],
                                             negnb2[:, hs], start=False, stop=True)
                        nc.vector.tensor_add(s_rows[:], scores8[:], gum8[:])
                        for pr in range(PAIRS):
                            nc.sync.dma_start(out=srow[pr][:], in_=s_rows[pr:pr + 1, :])

                        # nst[p, 8c+pr] = -s_rows[pr, 128c+p]
                        with tc.tile_pool(name="psum_st", bufs=1, space="PSUM") as pp_st:
                            st_ps = pp_st.tile([D, PAIRS * NCHUNK], F32)
                            for c in range(NCHUNK):
                                nc.tensor.transpose(
                                    st_ps[:, c * PAIRS:(c + 1) * PAIRS],
                                    s_rows[:, c * D:(c + 1) * D],
                                    ident[:PAIRS, :PAIRS],
                                )
                            nc.scalar.mul(nst[:], st_ps[:], -1.0)

                    # logits psum: s-part matmuls early (group stays open until
                    # the B-part accumulates at the end)
                    with tc.tile_pool(name="psum_l", bufs=1, space="PSUM") as pp_l, \
                         tc.tile_pool(name="psum_o", bufs=1, space="PSUM") as pp_o, \
                         tc.tile_pool(name="psum_bt", bufs=1, space="PSUM") as pp_bt:
                        logits = pp_l.tile([D, N], F32)
                        for h in range(2):
                            hs = slice(h * HALF, (h + 1) * HALF)
                            nc.tensor.matmul(logits[:, hs], lhs_s[:], s_rows[:, hs],
                                             start=True, stop=False)

                        # ---- B phase --------------------------------------------
                        with tc.tile_pool(name="bcast", bufs=3) as bc_pool, \
                             tc.tile_pool(name="scr", bufs=2) as scr_pool:
                            for pr in range(PAIRS):
                                bcast = bc_pool.tile([D, N], F32, tag="bcast")
                                nc.gpsimd.partition_broadcast(bcast[:], srow[pr][:])
                                pbase = pr * 32
                                for c in range(NCHUNK):
                                    bias_col = nst[:, c * PAIRS + pr: c * PAIRS + pr + 1]
                                    if c < ACT_SPLIT:
                                        scr = scr_pool.tile([D, N], F32, tag="scr_act")
                                        for g in range(2):
                                            nc.scalar.activation(
                                                out=scr[:, g * HALF:(g + 1) * HALF],
                                                in_=bcast[:, g * HALF:(g + 1) * HALF],
                                                func=AF.Abs, bias=bias_col, scale=1.0,
                                                accum_out=ptile[:, pbase + c * 4 + g:
                                                                pbase + c * 4 + g + 1],
                                            )
                                    else:
                                        scr = scr_pool.tile([D, N], F32, tag="scr_dve")
                                        nc.vector.tensor_scalar(
                                            scr[:], bcast[:], bias_col, None, ALU.add,
                                        )
                                        nc.vector.tensor_reduce(
                                            ptile[:, pbase + c * 4: pbase + c * 4 + 4],
                                            scr[:].rearrange("p (s f) -> p s f", s=4),
                                            AX.X, ALU.add, apply_absolute_value=True,
                                        )
                                # combine partials -> B columns for this pair
                                nc.vector.tensor_reduce(
                                    b_col[:, pr * NCHUNK:(pr + 1) * NCHUNK],
                                    ptile[:, pbase:pbase + 32].rearrange(
                                        "p (c g) -> p c g", g=4),
                                    AX.X, ALU.add,
                                )

                        # ---- B columns -> B rows --------------------------------
                        bt_ps = pp_bt.tile([PAIRS * NCHUNK, D], F32)
                        nc.tensor.transpose(bt_ps[:], b_col[:], ident[:])
                        nc.scalar.copy(bt_sb[:], bt_ps[:])
                        # flat orders line up: b_rows[pr, 128c+p] = bt_sb[8pr+c, p]
                        nc.sync.dma_start(out=b_rows[:], in_=bt_sb[:])

                        # ---- logits B-part, softmax, top-k sum ------------------
                        for h in range(2):
                            hs = slice(h * HALF, (h + 1) * HALF)
                            nc.tensor.matmul(logits[:, hs], lhs_b[:], b_rows[:, hs],
                                             start=False, stop=True)
                        nc.vector.tensor_reduce(negmax[:], logits[:], AX.X, ALU.max,
                                                negate=True)
                        nc.scalar.activation(out=e_sb[:], in_=logits[:], func=AF.Exp,
                                             bias=negmax[:], scale=1.0,
                                             accum_out=zden[:])
                        nc.vector.reciprocal(invz[:], zden[:])
                        nc.vector.tensor_scalar(p_sb[:], e_sb[:], invz[:], None, ALU.mult)

                        out_ps = pp_o.tile([PAIRS, N], F32)
                        for h in range(2):
                            hs = slice(h * HALF, (h + 1) * HALF)
                            nc.tensor.matmul(out_ps[:, hs], ones8t[:], p_sb[:, hs],
                                             start=True, stop=True)
                        nc.scalar.copy(out_sb[:], out_ps[:])
                        nc.sync.dma_start(out=d_out[:], in_=out_sb[:])

    nc.finalize()
    return nc


def host_inputs(query, neighbors, gumbel):
    """Per-core input maps. Core c handles pairs [8c, 8c+8)."""
    query = np.asarray(query, np.float32)
    neighbors = np.asarray(neighbors, np.float32)
    gumbel = np.asarray(gumbel, np.float32)

    nbT = np.ascontiguousarray(neighbors.T)                      # [128, 1024]
    negnb2 = -np.sum(neighbors * neighbors, 1)[None, :]          # [1, 1024]
    ident = np.eye(D, dtype=np.float32)
    ones8 = np.ones((1, PAIRS), np.float32)

    scaling = (N + 1 - 2 * np.arange(1, K + 1)).astype(np.float32)  # [16]
    lhs_s = np.zeros((PAIRS, D), np.float32)
    lhs_b = np.zeros((PAIRS, D), np.float32)
    ones8t = np.zeros((D, PAIRS), np.float32)
    for pr in range(PAIRS):
        lhs_s[pr, 16 * pr:16 * pr + K] = scaling
        lhs_b[pr, 16 * pr:16 * pr + K] = -1.0
        ones8t[16 * pr:16 * pr + K, pr] = 1.0

    gflat = gumbel.reshape(S * M, N)
    in_maps = []
    for c in range(NCORES):
        m0 = (PAIRS * c) % M
        in_maps.append({
            "nbT": nbT,
            "qT2": np.ascontiguousarray(2.0 * query.T[:, m0:m0 + PAIRS]),
            "negnb2": negnb2,
            "gum8": np.ascontiguousarray(gflat[PAIRS * c:PAIRS * (c + 1)]),
            "ident": ident,
            "ones8": ones8,
            "lhs_s": lhs_s,
            "lhs_b": lhs_b,
            "ones8t": ones8t,
        })
    return in_maps


_NC_CACHE = {}


def _get_nc():
    if "nc" not in _NC_CACHE:
        _NC_CACHE["nc"] = build_nc()
    return _NC_CACHE["nc"]


def run(query, neighbors, gumbel, trace=False):
    nc = _get_nc()
    in_maps = host_inputs(query, neighbors, gumbel)
    res = run_bass_kernel_spmd(nc, in_maps, list(range(NCORES)), trace=trace)
    outs = np.stack([res.results[c]["out"] for c in range(NCORES)])  # [8, 8, 1024]
    full = outs.reshape(S, M, N).astype(np.float32)
    return full, res


def kernel(query, neighbors, gumbel):
    full, _ = run(query, neighbors, gumbel, trace=False)
    return full


def _numpy_model(query, neighbors, gumbel):
    """Host model of what the device computes (for sim validation)."""
    q = np.asarray(query, np.float32)
    nb = np.asarray(neighbors, np.float32)
    g = np.asarray(gumbel, np.float32).reshape(S * M, N)
    t = 2.0 * q @ nb.T - np.sum(nb * nb, 1)[None, :]    # [32, 1024]
    t = np.concatenate([t, t], 0)                       # [64, 1024] (p-major)
    s = t + g
    B = np.abs(s[:, :, None] - s[:, None, :]).sum(2)    # [64, 1024]
    scaling = (N + 1 - 2 * np.arange(1, K + 1)).astype(np.float32)
    l = scaling[None, :, None] * s[:, None, :] - B[:, None, :]  # [64, 16, 1024]
    l = l - l.max(2, keepdims=True)
    e = np.exp(l)
    p = e / e.sum(2, keepdims=True)
    return p.sum(1).reshape(S, M, N)


def _selftest_sim():
    """Validate core 0 under CoreSim against the numpy model."""
    from concourse.bass_interp import CoreSim

    rng = np.random.default_rng(0)
    query = rng.normal(size=(M, D)).astype(np.float32)
    neighbors = rng.normal(size=(N, D)).astype(np.float32)
    u = rng.uniform(1e-6, 1 - 1e-6, size=(S, M, N)).astype(np.float32)
    gumbel = -np.log(-np.log(u)).astype(np.float32)

    nc = _get_nc()
    in_maps = host_inputs(query, neighbors, gumbel)
    sim = CoreSim(nc)
    for k, v in in_maps[0].items():
        sim.tensor(k)[:] = v
    sim.simulate()
    got = np.array(sim.tensor("out"))
    want = _numpy_model(query, neighbors, gumbel).reshape(S * M, N)[:PAIRS]
    err = np.linalg.norm(got - want) / np.linalg.norm(want)
    print("sim rel err:", err)
    print("sim time (model ns):", sim.time)
    assert err < 2e-2, err
    print("SIM PASS")


if __name__ == "__main__":
    if "--sim" in sys.argv:
        _selftest_sim()


# revision 9
# speedup vs baseline: 1.0507x; 1.0507x over previous
"""
Trainium2 Bass kernel for nn_DKNN (differentiable kNN via NeuralSort + PL sampling).

Math (per (sample p, query m) pair, n=1024 neighbors, K=16, tau=1):
    scores[m,n] = -||q_m - nb_n||^2 ; softmax over n is invariant to the
    per-query ||q_m||^2 term, so we use  t = 2 q.nb - ||nb||^2  instead.
    s = t + gumbel                      # [n]
    B_i = sum_j |s_i - s_j|             # O(n^2) -- the hot loop
    l[r,i] = scaling_r * s_i - B_i      # scaling_r = n+1-2(r+1), r=0..15
    out[i] = sum_r softmax_i(l[r,:])    # [n]

Sharding: 64 independent (p,m) pairs -> 8 pairs per NeuronCore.

Engine mapping per core (v2):
    GPS   : partition_broadcast of each pair's s row -> SBUF [128,1024]
    ACT   : fused |bcast - s_i| + accumulate, segmented FD=512 for fp32
            accuracy (B partial sums must not run 1024-long sequentially);
            exp with bias=-rowmax and accumulate -> Z
    DVE   : other strips: tensor_scalar subtract (2x) + segmented abs-reduce
            [128,4,256]; rowmax; reciprocal; P = E * (1/Z); combines
    PE    : scores matmul, s transposes, logits outer-product matmuls
            (s-part early / B-part late), final K-row-sum matmul
"""

import os
import sys

import numpy as np

sys.path.insert(0, "/opt/trn_rl_repo")


def _install_ntff_hook_shim():
    """The agent image's `antenv` lacks `axon_hooks`; provide it so
    run_bass_kernel_spmd(trace=True) can capture NTFF profiles via the
    boot module's ctypes hook."""
    import types

    if "antenv.axon_hooks" in sys.modules:
        return
    mod = types.ModuleType("antenv.axon_hooks")
    state = {"hook": None}
    mod.set_axon_ntff_profile_hook = lambda h: state.__setitem__("hook", h)
    mod.get_axon_ntff_profile_hook = lambda: state["hook"]
    sys.modules["antenv.axon_hooks"] = mod
    try:
        from trn_agent_boot.trn_boot import _ntff_profile_via_ctypes

        mod.set_axon_ntff_profile_hook(
            _ntff_profile_via_ctypes("/opt/axon/libaxon_pjrt.so")
        )
    except Exception:
        pass


_install_ntff_hook_shim()

import concourse.bass as bass
import concourse.mybir as mybir
import concourse.tile as tile
from concourse import bacc
from concourse.bass_utils import run_bass_kernel_spmd

F32 = mybir.dt.float32
AF = mybir.ActivationFunctionType
ALU = mybir.AluOpType
AX = mybir.AxisListType

N = 1024          # neighbors
D = 128           # feature dim
M = 32            # queries
S = 2             # PL samples
K = 16            # top-k
NCORES = 8
PAIRS = 8         # (p, m) pairs per core
NCHUNK = 8        # i-chunks of 128 per pair
HALF = 512        # matmul N <= 512 (one PSUM bank)

# strips (i-chunks) 0..ACT_SPLIT-1 on ScalarE, the rest on VectorE
ACT_SPLIT = int(os.environ.get("DK_ACT_SPLIT", "4"))


def build_nc():
    nc = bacc.Bacc("TRN2", target_bir_lowering=False, debug=False)

    with tile.TileContext(nc) as tc:
        with tc.tile_pool(name="dram", bufs=1, space="DRAM") as dram:
            d_nbT = dram.tile([D, N], F32, kind="ExternalInput", name="nbT", uniquify=False)
            d_qT2 = dram.tile([D, PAIRS], F32, kind="ExternalInput", name="qT2", uniquify=False)
            d_gum8 = dram.tile([PAIRS, N], F32, kind="ExternalInput", name="gum8", uniquify=False)
            d_ident = dram.tile([D, D], F32, kind="ExternalInput", name="ident", uniquify=False)
            d_lhs_sb = dram.tile([2 * PAIRS, D], F32, kind="ExternalInput", name="lhs_sb", uniquify=False)
            d_ones8t = dram.tile([D, PAIRS], F32, kind="ExternalInput", name="ones8t", uniquify=False)
            d_out = dram.tile([PAIRS, N], F32, kind="ExternalOutput", name="out", uniquify=False)

            with tc.tile_pool(name="consts", bufs=1) as consts:
                nbT = consts.tile([D, N], F32)
                qT2 = consts.tile([D, PAIRS], F32)
                gum8 = consts.tile([PAIRS, N], F32)
                ident = consts.tile([D, D], F32)
                lhs_sb = consts.tile([2 * PAIRS, D], F32)
                ones8t = consts.tile([D, PAIRS], F32)
                # spread input loads across DMA queues
                nc.sync.dma_start(out=nbT[:], in_=d_nbT[:])
                nc.scalar.dma_start(out=qT2[:], in_=d_qT2[:])
                nc.scalar.dma_start(out=gum8[:], in_=d_gum8[:])
                nc.gpsimd.dma_start(out=ident[:], in_=d_ident[:])
                nc.gpsimd.dma_start(out=lhs_sb[:], in_=d_lhs_sb[:])
                nc.gpsimd.dma_start(out=ones8t[:], in_=d_ones8t[:])

                with tc.tile_pool(name="work", bufs=1) as work:
                    sb_rows = work.tile([2 * PAIRS, N], F32)  # rows 0-7: s, rows 8-15: B
                    s_rows = sb_rows[0:PAIRS, :]
                    nst = work.tile([D, PAIRS * NCHUNK], F32)   # col 8c+pr = -s_pr[128c+p]
                    ptile = work.tile([D, PAIRS * 32], F32)     # col pr*32+c*4+g = partial sums
                    b_col = work.tile([D, PAIRS * NCHUNK], F32)  # col 8pr+c = B_pr[128c+p]
                    bt_sb = work.tile([PAIRS * NCHUNK, D], F32)
                    e_sb = work.tile([D, N], F32)
                    p_sb = work.tile([D, N], F32)
                    negmax = work.tile([D, 1], F32)
                    zden = work.tile([D, 1], F32)
                    invz = work.tile([D, 1], F32)
                    out_sb = work.tile([PAIRS, N], F32)
                    srow = [work.tile([1, N], F32, name=f"srow{i}") for i in range(PAIRS)]

                    nc.gpsimd.memset(ptile[:], 0.0)

                    # ---- s = (2 q.nb - nb2) + gumbel -----------------------------
                    with tc.tile_pool(name="psum_s", bufs=1, space="PSUM") as pp_s:
                        scores8 = pp_s.tile([PAIRS, N], F32)
                        for h in range(2):
                            hs = slice(h * HALF, (h + 1) * HALF)
                            nc.tensor.matmul(scores8[:, hs], qT2[:], nbT[:, hs],
                                             start=True, stop=True)
                        nc.vector.tensor_add(s_rows, scores8[:], gum8[:])
                        for pr in range(PAIRS):
                            eng = [nc.sync, nc.scalar][pr % 2]
                            eng.dma_start(out=srow[pr][:], in_=sb_rows[pr:pr + 1, :])

                        # nst[p, 8c+pr] = -s_rows[pr, 128c+p]
                        with tc.tile_pool(name="psum_st", bufs=1, space="PSUM") as pp_st:
                            st_ps = pp_st.tile([D, PAIRS * NCHUNK], F32)
                            for c in range(NCHUNK):
                                nc.tensor.transpose(
                                    st_ps[:, c * PAIRS:(c + 1) * PAIRS],
                                    sb_rows[0:PAIRS, c * D:(c + 1) * D],
                                    ident[:PAIRS, :PAIRS],
                                )
                            nc.scalar.mul(nst[:], st_ps[:], -1.0)

                    # logits psum: s-part matmuls early (group stays open until
                    # the B-part accumulates at the end)
                    with tc.tile_pool(name="psum_l", bufs=1, space="PSUM") as pp_l, \
                         tc.tile_pool(name="psum_o", bufs=1, space="PSUM") as pp_o, \
                         tc.tile_pool(name="psum_bt", bufs=1, space="PSUM") as pp_bt:
                        logits = pp_l.tile([D, N], F32)

                        # ---- B phase --------------------------------------------
                        with tc.tile_pool(name="bcast", bufs=3) as bc_pool, \
                             tc.tile_pool(name="scr", bufs=2) as scr_pool:
                            for pr in range(PAIRS):
                                bcast = bc_pool.tile([D, N], F32, tag="bcast")
                                nc.gpsimd.partition_broadcast(bcast[:], srow[pr][:])
                                pbase = pr * 32
                                for c in range(NCHUNK):
                                    bias_col = nst[:, c * PAIRS + pr: c * PAIRS + pr + 1]
                                    if c < ACT_SPLIT:
                                        scr = scr_pool.tile([D, N], F32, tag="scr_act")
                                        for g in range(2):
                                            nc.scalar.activation(
                                                out=scr[:, g * HALF:(g + 1) * HALF],
                                                in_=bcast[:, g * HALF:(g + 1) * HALF],
                                                func=AF.Abs, bias=bias_col, scale=1.0,
                                                accum_out=ptile[:, pbase + c * 4 + g:
                                                                pbase + c * 4 + g + 1],
                                            )
                                    else:
                                        scr = scr_pool.tile([D, N], F32, tag="scr_dve")
                                        nc.vector.tensor_scalar(
                                            scr[:], bcast[:], bias_col, None, ALU.add,
                                        )
                                        nc.vector.tensor_reduce(
                                            ptile[:, pbase + c * 4: pbase + c * 4 + 4],
                                            scr[:].rearrange("p (s f) -> p s f", s=4),
                                            AX.X, ALU.add, apply_absolute_value=True,
                                        )
                                # combine partials -> B columns for this pair
                                nc.vector.tensor_reduce(
                                    b_col[:, pr * NCHUNK:(pr + 1) * NCHUNK],
                                    ptile[:, pbase:pbase + 32].rearrange(
                                        "p (c g) -> p c g", g=4),
                                    AX.X, ALU.add,
                                )

                        # ---- B columns -> B rows --------------------------------
                        bt_ps = pp_bt.tile([PAIRS * NCHUNK, D], F32)
                        nc.tensor.transpose(bt_ps[:], b_col[:], ident[:])
                        nc.scalar.copy(bt_sb[:], bt_ps[:])
                        # flat orders line up: B_rows[pr, 128c+p] = bt_sb[8pr+c, p]
                        nc.sync.dma_start(out=sb_rows[PAIRS:2 * PAIRS, :], in_=bt_sb[:])

                        # ---- logits, softmax, top-k sum -------------------------
                        for h in range(2):
                            hs = slice(h * HALF, (h + 1) * HALF)
                            nc.tensor.matmul(logits[:, hs], lhs_sb[:], sb_rows[:, hs],
                                             start=True, stop=True)
                        nc.vector.tensor_reduce(negmax[:], logits[:], AX.X, ALU.max,
                                                negate=True)
                        nc.scalar.activation(out=e_sb[:], in_=logits[:], func=AF.Exp,
                                             bias=negmax[:], scale=1.0,
                                             accum_out=zden[:])
                        nc.vector.reciprocal(invz[:], zden[:])
                        nc.vector.tensor_scalar(p_sb[:], e_sb[:], invz[:], None, ALU.mult)

                        out_ps = pp_o.tile([PAIRS, N], F32)
                        for h in range(2):
                            hs = slice(h * HALF, (h + 1) * HALF)
                            nc.tensor.matmul(out_ps[:, hs], ones8t[:], p_sb[:, hs],
                                             start=True, stop=True)
                        nc.scalar.copy(out_sb[:], out_ps[:])
                        nc.sync.dma_start(out=d_out[:], in_=out_sb[:])

    nc.finalize()
    return nc


def host_inputs(query, neighbors, gumbel):
    """Per-core input maps. Core c handles pairs [8c, 8c+8)."""
    query = np.asarray(query, np.float32)
    neighbors = np.asarray(neighbors, np.float32)
    gumbel = np.asarray(gumbel, np.float32)

    nbT = np.ascontiguousarray(neighbors.T)                      # [128, 1024]
    nb2 = np.sum(neighbors * neighbors, 1)[None, :]              # [1, 1024]
    ident = np.eye(D, dtype=np.float32)

    scaling = (N + 1 - 2 * np.arange(1, K + 1)).astype(np.float32)  # [16]
    lhs_sb = np.zeros((2 * PAIRS, D), np.float32)
    ones8t = np.zeros((D, PAIRS), np.float32)
    for pr in range(PAIRS):
        lhs_sb[pr, 16 * pr:16 * pr + K] = scaling
        lhs_sb[PAIRS + pr, 16 * pr:16 * pr + K] = -1.0
        ones8t[16 * pr:16 * pr + K, pr] = 1.0

    gflat = gumbel.reshape(S * M, N)
    in_maps = []
    for c in range(NCORES):
        m0 = (PAIRS * c) % M
        in_maps.append({
            "nbT": nbT,
            "qT2": np.ascontiguousarray(2.0 * query.T[:, m0:m0 + PAIRS]),
            "gum8": np.ascontiguousarray(gflat[PAIRS * c:PAIRS * (c + 1)] - nb2),
            "ident": ident,
            "lhs_sb": lhs_sb,
            "ones8t": ones8t,
        })
    return in_maps


_NC_CACHE = {}


def _get_nc():
    if "nc" not in _NC_CACHE:
        _NC_CACHE["nc"] = build_nc()
    return _NC_CACHE["nc"]


def run(query, neighbors, gumbel, trace=False):
    nc = _get_nc()
    in_maps = host_inputs(query, neighbors, gumbel)
    res = run_bass_kernel_spmd(nc, in_maps, list(range(NCORES)), trace=trace)
    outs = np.stack([res.results[c]["out"] for c in range(NCORES)])  # [8, 8, 1024]
    full = outs.reshape(S, M, N).astype(np.float32)
    return full, res


def kernel(query, neighbors, gumbel):
    full, _ = run(query, neighbors, gumbel, trace=False)
    return full


def _numpy_model(query, neighbors, gumbel):
    """Host model of what the device computes (for sim validation)."""
    q = np.asarray(query, np.float32)
    nb = np.asarray(neighbors, np.float32)
    g = np.asarray(gumbel, np.float32).reshape(S * M, N)
    t = 2.0 * q @ nb.T - np.sum(nb * nb, 1)[None, :]    # [32, 1024]
    t = np.concatenate([t, t], 0)                       # [64, 1024] (p-major)
    s = t + g
    B = np.abs(s[:, :, None] - s[:, None, :]).sum(2)    # [64, 1024]
    scaling = (N + 1 - 2 * np.arange(1, K + 1)).astype(np.float32)
    l = scaling[None, :, None] * s[:, None, :] - B[:, None, :]  # [64, 16, 1024]
    l = l - l.max(2, keepdims=True)
    e = np.exp(l)
    p = e / e.sum(2, keepdims=True)
    return p.sum(1).reshape(S, M, N)


def _selftest_sim():
    """Validate core 0 under CoreSim against the numpy model."""
    from concourse.bass_interp import CoreSim

    rng = np.random.default_rng(0)
    query = rng.normal(size=(M, D)).astype(np.float32)
    neighbors = rng.normal(size=(N, D)).astype(np.float32)
    u = rng.uniform(1e-6, 1 - 1e-6, size=(S, M, N)).astype(np.float32)
    gumbel = -np.log(-np.log(u)).astype(np.float32)

    nc = _get_nc()
    in_maps = host_inputs(query, neighbors, gumbel)
    sim = CoreSim(nc)
    for k, v in in_maps[0].items():
        sim.tensor(k)[:] = v
    sim.simulate()
    got = np.array(sim.tensor("out"))
    want = _numpy_model(query, neighbors, gumbel).reshape(S * M, N)[:PAIRS]
    err = np.linalg.norm(got - want) / np.linalg.norm(want)
    print("sim rel err:", err)
    print("sim time (model ns):", sim.time)
    assert err < 2e-2, err
    print("SIM PASS")


if __name__ == "__main__":
    if "--sim" in sys.argv:
        _selftest_sim()


# revision 10
# speedup vs baseline: 1.0779x; 1.0259x over previous
"""
Trainium2 Bass kernel for nn_DKNN (differentiable kNN via NeuralSort + PL sampling).

Math (per (sample p, query m) pair, n=1024 neighbors, K=16, tau=1):
    scores[m,n] = -||q_m - nb_n||^2 ; softmax over n is invariant to the
    per-query ||q_m||^2 term, so we use  t = 2 q.nb - ||nb||^2  instead.
    s = t + gumbel                      # [n]
    B_i = sum_j |s_i - s_j|             # O(n^2) -- the hot loop
    l[r,i] = scaling_r * s_i - B_i      # scaling_r = n+1-2(r+1), r=0..15
    out[i] = sum_r softmax_i(l[r,:])    # [n]

Sharding: 64 independent (p,m) pairs -> 8 pairs per NeuronCore.

Engine mapping per core (v2):
    GPS   : partition_broadcast of each pair's s row -> SBUF [128,1024]
    ACT   : fused |bcast - s_i| + accumulate, segmented FD=512 for fp32
            accuracy (B partial sums must not run 1024-long sequentially);
            exp with bias=-rowmax and accumulate -> Z
    DVE   : other strips: tensor_scalar subtract (2x) + segmented abs-reduce
            [128,4,256]; rowmax; reciprocal; P = E * (1/Z); combines
    PE    : scores matmul, s transposes, logits outer-product matmuls
            (s-part early / B-part late), final K-row-sum matmul
"""

import os
import sys

import numpy as np

sys.path.insert(0, "/opt/trn_rl_repo")


def _install_ntff_hook_shim():
    """The agent image's `antenv` lacks `axon_hooks`; provide it so
    run_bass_kernel_spmd(trace=True) can capture NTFF profiles via the
    boot module's ctypes hook."""
    import types

    if "antenv.axon_hooks" in sys.modules:
        return
    mod = types.ModuleType("antenv.axon_hooks")
    state = {"hook": None}
    mod.set_axon_ntff_profile_hook = lambda h: state.__setitem__("hook", h)
    mod.get_axon_ntff_profile_hook = lambda: state["hook"]
    sys.modules["antenv.axon_hooks"] = mod
    try:
        from trn_agent_boot.trn_boot import _ntff_profile_via_ctypes

        mod.set_axon_ntff_profile_hook(
            _ntff_profile_via_ctypes("/opt/axon/libaxon_pjrt.so")
        )
    except Exception:
        pass


_install_ntff_hook_shim()

import concourse.bass as bass
import concourse.mybir as mybir
import concourse.tile as tile
from concourse import bacc
from concourse.bass_utils import run_bass_kernel_spmd

F32 = mybir.dt.float32
F16 = mybir.dt.float16
AF = mybir.ActivationFunctionType
ALU = mybir.AluOpType
AX = mybir.AxisListType

N = 1024          # neighbors
D = 128           # feature dim
M = 32            # queries
S = 2             # PL samples
K = 16            # top-k
NCORES = 8
PAIRS = 8         # (p, m) pairs per core
NCHUNK = 8        # i-chunks of 128 per pair
HALF = 512        # matmul N <= 512 (one PSUM bank)

# strips (i-chunks) 0..ACT_SPLIT-1 on ScalarE, the rest on VectorE
ACT_SPLIT = int(os.environ.get("DK_ACT_SPLIT", "4"))


def build_nc():
    nc = bacc.Bacc("TRN2", target_bir_lowering=False, debug=False)

    with tile.TileContext(nc) as tc:
        with tc.tile_pool(name="dram", bufs=1, space="DRAM") as dram:
            d_nbT = dram.tile([D, N], F32, kind="ExternalInput", name="nbT", uniquify=False)
            d_qT2 = dram.tile([D, PAIRS], F32, kind="ExternalInput", name="qT2", uniquify=False)
            d_gum8 = dram.tile([PAIRS, N], F32, kind="ExternalInput", name="gum8", uniquify=False)
            d_ident = dram.tile([D, D], F32, kind="ExternalInput", name="ident", uniquify=False)
            d_lhs_sb = dram.tile([2 * PAIRS, D], F32, kind="ExternalInput", name="lhs_sb", uniquify=False)
            d_ones8t = dram.tile([D, PAIRS], F32, kind="ExternalInput", name="ones8t", uniquify=False)
            d_out = dram.tile([PAIRS, N], F32, kind="ExternalOutput", name="out", uniquify=False)

            with tc.tile_pool(name="consts", bufs=1) as consts:
                nbT = consts.tile([D, N], F32)
                qT2 = consts.tile([D, PAIRS], F32)
                gum8 = consts.tile([PAIRS, N], F32)
                ident = consts.tile([D, D], F32)
                lhs_sb = consts.tile([2 * PAIRS, D], F32)
                ones8t = consts.tile([D, PAIRS], F32)
                # spread input loads across DMA queues
                nc.sync.dma_start(out=nbT[:, 0:HALF], in_=d_nbT[:, 0:HALF])
                nc.scalar.dma_start(out=nbT[:, HALF:N], in_=d_nbT[:, HALF:N])
                nc.sync.dma_start(out=qT2[:], in_=d_qT2[:])
                nc.scalar.dma_start(out=gum8[:], in_=d_gum8[:])
                nc.gpsimd.dma_start(out=ident[:], in_=d_ident[:])
                nc.gpsimd.dma_start(out=lhs_sb[:], in_=d_lhs_sb[:])
                nc.gpsimd.dma_start(out=ones8t[:], in_=d_ones8t[:])

                with tc.tile_pool(name="work", bufs=1) as work:
                    sb_rows = work.tile([2 * PAIRS, N], F32)  # rows 0-7: s, rows 8-15: B
                    s_rows = sb_rows[0:PAIRS, :]
                    nst = work.tile([D, PAIRS * NCHUNK], F32)   # col 8c+pr = -s_pr[128c+p]
                    ptile = work.tile([D, PAIRS * 32], F32)     # col pr*32+c*4+g = partial sums
                    b_col = work.tile([D, PAIRS * NCHUNK], F32)  # col 8pr+c = B_pr[128c+p]
                    bt_sb = work.tile([PAIRS * NCHUNK, D], F32)
                    e_sb = work.tile([D, N], F16)
                    p_sb = work.tile([D, N], F16)
                    ones8t16 = work.tile([D, PAIRS], F16)
                    negmax = work.tile([D, 1], F32)
                    zden = work.tile([D, 1], F32)
                    invz = work.tile([D, 1], F32)
                    out_sb = work.tile([PAIRS, N], F32)
                    srow = [work.tile([1, N], F32, name=f"srow{i}") for i in range(PAIRS)]

                    nc.gpsimd.memset(ptile[:], 0.0)
                    nc.vector.tensor_copy(ones8t16[:], ones8t[:])

                    # ---- s = (2 q.nb - nb2) + gumbel -----------------------------
                    with tc.tile_pool(name="psum_s", bufs=1, space="PSUM") as pp_s:
                        scores8 = pp_s.tile([PAIRS, N], F32)
                        for h in range(2):
                            hs = slice(h * HALF, (h + 1) * HALF)
                            nc.tensor.matmul(scores8[:, hs], qT2[:], nbT[:, hs],
                                             start=True, stop=True)
                        nc.vector.tensor_add(s_rows, scores8[:], gum8[:])
                        for pr in range(PAIRS):
                            eng = [nc.sync, nc.scalar][pr % 2]
                            eng.dma_start(out=srow[pr][:], in_=sb_rows[pr:pr + 1, :])

                        # nst[p, 8c+pr] = -s_rows[pr, 128c+p]
                        with tc.tile_pool(name="psum_st", bufs=1, space="PSUM") as pp_st:
                            st_ps = pp_st.tile([D, PAIRS * NCHUNK], F32)
                            for half_c in range(2):
                                for c in range(half_c * 4, half_c * 4 + 4):
                                    nc.tensor.transpose(
                                        st_ps[:, c * PAIRS:(c + 1) * PAIRS],
                                        sb_rows[0:PAIRS, c * D:(c + 1) * D],
                                        ident[:PAIRS, :PAIRS],
                                    )
                                sl = slice(half_c * 4 * PAIRS, (half_c * 4 + 4) * PAIRS)
                                nc.scalar.mul(nst[:, sl], st_ps[:, sl], -1.0)

                    # logits psum: s-part matmuls early (group stays open until
                    # the B-part accumulates at the end)
                    with tc.tile_pool(name="psum_l", bufs=1, space="PSUM") as pp_l, \
                         tc.tile_pool(name="psum_o", bufs=1, space="PSUM") as pp_o, \
                         tc.tile_pool(name="psum_bt", bufs=1, space="PSUM") as pp_bt:
                        logits = pp_l.tile([D, N], F32)

                        # ---- B phase --------------------------------------------
                        with tc.tile_pool(name="bcast", bufs=3) as bc_pool, \
                             tc.tile_pool(name="scr", bufs=2) as scr_pool:
                            for pr in range(PAIRS):
                                bcast = bc_pool.tile([D, N], F32, tag="bcast")
                                nc.gpsimd.partition_broadcast(bcast[:], srow[pr][:])
                                pbase = pr * 32
                                for c in range(NCHUNK):
                                    bias_col = nst[:, c * PAIRS + pr: c * PAIRS + pr + 1]
                                    if c < ACT_SPLIT:
                                        scr = scr_pool.tile([D, N], F32, tag="scr_act")
                                        for g in range(2):
                                            nc.scalar.activation(
                                                out=scr[:, g * HALF:(g + 1) * HALF],
                                                in_=bcast[:, g * HALF:(g + 1) * HALF],
                                                func=AF.Abs, bias=bias_col, scale=1.0,
                                                accum_out=ptile[:, pbase + c * 4 + g:
                                                                pbase + c * 4 + g + 1],
                                            )
                                    else:
                                        scr = scr_pool.tile([D, N], F32, tag="scr_dve")
                                        nc.vector.tensor_scalar(
                                            scr[:], bcast[:], bias_col, None, ALU.add,
                                        )
                                        nc.vector.tensor_reduce(
                                            ptile[:, pbase + c * 4: pbase + c * 4 + 4],
                                            scr[:].rearrange("p (s f) -> p s f", s=4),
                                            AX.X, ALU.add, apply_absolute_value=True,
                                        )
                                # combine partials -> B columns for this pair
                                nc.vector.tensor_reduce(
                                    b_col[:, pr * NCHUNK:(pr + 1) * NCHUNK],
                                    ptile[:, pbase:pbase + 32].rearrange(
                                        "p (c g) -> p c g", g=4),
                                    AX.X, ALU.add,
                                )

                        # ---- B columns -> B rows --------------------------------
                        bt_ps = pp_bt.tile([PAIRS * NCHUNK, D], F32)
                        nc.tensor.transpose(bt_ps[:], b_col[:], ident[:])
                        nc.scalar.copy(bt_sb[:], bt_ps[:])
                        # flat orders line up: B_rows[pr, 128c+p] = bt_sb[8pr+c, p]
                        nc.sync.dma_start(out=sb_rows[PAIRS:2 * PAIRS, :], in_=bt_sb[:])

                        # ---- logits, softmax, top-k sum -------------------------
                        for h in range(2):
                            hs = slice(h * HALF, (h + 1) * HALF)
                            nc.tensor.matmul(logits[:, hs], lhs_sb[:], sb_rows[:, hs],
                                             start=True, stop=True)
                        nc.vector.tensor_reduce(negmax[:], logits[:], AX.X, ALU.max,
                                                negate=True)
                        nc.scalar.activation(out=e_sb[:], in_=logits[:], func=AF.Exp,
                                             bias=negmax[:], scale=1.0,
                                             accum_out=zden[:])
                        nc.vector.reciprocal(invz[:], zden[:])
                        nc.vector.tensor_scalar(p_sb[:], e_sb[:], invz[:], None, ALU.mult)

                        out_ps = pp_o.tile([PAIRS, N], F32)
                        for h in range(2):
                            hs = slice(h * HALF, (h + 1) * HALF)
                            nc.tensor.matmul(out_ps[:, hs], ones8t16[:], p_sb[:, hs],
                                             start=True, stop=True)
                        nc.scalar.copy(out_sb[:], out_ps[:])
                        nc.sync.dma_start(out=d_out[:], in_=out_sb[:])

    nc.finalize()
    return nc


def host_inputs(query, neighbors, gumbel):
    """Per-core input maps. Core c handles pairs [8c, 8c+8)."""
    query = np.asarray(query, np.float32)
    neighbors = np.asarray(neighbors, np.float32)
    gumbel = np.asarray(gumbel, np.float32)

    nbT = np.ascontiguousarray(neighbors.T)                      # [128, 1024]
    nb2 = np.sum(neighbors * neighbors, 1)[None, :]              # [1, 1024]
    ident = np.eye(D, dtype=np.float32)

    scaling = (N + 1 - 2 * np.arange(1, K + 1)).astype(np.float32)  # [16]
    lhs_sb = np.zeros((2 * PAIRS, D), np.float32)
    ones8t = np.zeros((D, PAIRS), np.float32)
    for pr in range(PAIRS):
        lhs_sb[pr, 16 * pr:16 * pr + K] = scaling
        lhs_sb[PAIRS + pr, 16 * pr:16 * pr + K] = -1.0
        ones8t[16 * pr:16 * pr + K, pr] = 1.0

    gflat = gumbel.reshape(S * M, N)
    in_maps = []
    for c in range(NCORES):
        m0 = (PAIRS * c) % M
        in_maps.append({
            "nbT": nbT,
            "qT2": np.ascontiguousarray(2.0 * query.T[:, m0:m0 + PAIRS]),
            "gum8": np.ascontiguousarray(gflat[PAIRS * c:PAIRS * (c + 1)] - nb2),
            "ident": ident,
            "lhs_sb": lhs_sb,
            "ones8t": ones8t,
        })
    return in_maps


_NC_CACHE = {}


def _get_nc():
    if "nc" not in _NC_CACHE:
        _NC_CACHE["nc"] = build_nc()
    return _NC_CACHE["nc"]


def run(query, neighbors, gumbel, trace=False):
    nc = _get_nc()
    in_maps = host_inputs(query, neighbors, gumbel)
    res = run_bass_kernel_spmd(nc, in_maps, list(range(NCORES)), trace=trace)
    outs = np.stack([res.results[c]["out"] for c in range(NCORES)])  # [8, 8, 1024]
    full = outs.reshape(S, M, N).astype(np.float32)
    return full, res


def kernel(query, neighbors, gumbel):
    full, _ = run(query, neighbors, gumbel, trace=False)
    return full


def _numpy_model(query, neighbors, gumbel):
    """Host model of what the device computes (for sim validation)."""
    q = np.asarray(query, np.float32)
    nb = np.asarray(neighbors, np.float32)
    g = np.asarray(gumbel, np.float32).reshape(S * M, N)
    t = 2.0 * q @ nb.T - np.sum(nb * nb, 1)[None, :]    # [32, 1024]
    t = np.concatenate([t, t], 0)                       # [64, 1024] (p-major)
    s = t + g
    B = np.abs(s[:, :, None] - s[:, None, :]).sum(2)    # [64, 1024]
    scaling = (N + 1 - 2 * np.arange(1, K + 1)).astype(np.float32)
    l = scaling[None, :, None] * s[:, None, :] - B[:, None, :]  # [64, 16, 1024]
    l = l - l.max(2, keepdims=True)
    e = np.exp(l)
    p = e / e.sum(2, keepdims=True)
    return p.sum(1).reshape(S, M, N)


def _selftest_sim():
    """Validate core 0 under CoreSim against the numpy model."""
    from concourse.bass_interp import CoreSim

    rng = np.random.default_rng(0)
    query = rng.normal(size=(M, D)).astype(np.float32)
    neighbors = rng.normal(size=(N, D)).astype(np.float32)
    u = rng.uniform(1e-6, 1 - 1e-6, size=(S, M, N)).astype(np.float32)
    gumbel = -np.log(-np.log(u)).astype(np.float32)

    nc = _get_nc()
    in_maps = host_inputs(query, neighbors, gumbel)
    sim = CoreSim(nc)
    for k, v in in_maps[0].items():
        sim.tensor(k)[:] = v
    sim.simulate()
    got = np.array(sim.tensor("out"))
    want = _numpy_model(query, neighbors, gumbel).reshape(S * M, N)[:PAIRS]
    err = np.linalg.norm(got - want) / np.linalg.norm(want)
    print("sim rel err:", err)
    print("sim time (model ns):", sim.time)
    assert err < 2e-2, err
    print("SIM PASS")


if __name__ == "__main__":
    if "--sim" in sys.argv:
        _selftest_sim()
